# revision 1
# baseline (speedup 1.0000x reference)
"""DeepSeek-V3 MoE (T=4096,H=2048,E=32,I=1024,IS=2048, top8/32 grouped routing)
on 8 trn2 NeuronCores.

Strategy (expert-parallel per sharding hint):
- core c owns experts 4c..4c+3 (dense compute over all tokens) + a 256-wide
  tensor-parallel slice of the shared expert's intermediate dim.
- Routing (gate gemm + sigmoid + grouped top-k) is computed on host in fp32 --
  it is 0.03% of FLOPs but selection must match the reference bit-for-bit;
  device top-k via approximate ACT sigmoid tables would flip near-tie experts.
  Combine weights are shipped as a per-token scale, folded into the expert
  intermediate activations (so PSUM accumulates the weighted combine).
- All big gemms run on the PE in float32r (full-rate fp32, ~1.4e-4 rms err).
- Host sums the 8 per-core partial outputs (the "all-reduce").

Layout on device: features on partitions, tokens on the free dim.
  xT [H, T] -> hT_e = silu(w1_e @ x^T) * (w3_e @ x^T) * colw_e -> [I, T]
  yT += w2_e^T^T... out[h', t] accumulated over (expert, i) in PSUM groups.
"""

import numpy as np

import concourse.bass as bass
import concourse.mybir as mybir
import concourse.tile as tile
from concourse import bacc
from concourse.bass_utils import run_bass_kernel_spmd

F32 = mybir.dt.float32
F32R = mybir.dt.float32r
AF = mybir.ActivationFunctionType

# Problem constants
TOP_K, N_GROUP, TOPK_GROUP, ROUTED_SCALE = 8, 8, 4, 2.5
T, H, E, I, IS = 4096, 2048, 32, 1024, 2048
N_CORES = 8
EL = E // N_CORES          # local experts per core = 4
ISL = IS // N_CORES        # shared-expert intermediate slice = 256

TB = 512                   # token block (free dim of matmuls / PSUM width)
HT = H // 128              # 16 h-tiles
IT = I // 128              # 8 i-tiles per routed expert
IST = ISL // 128           # 2 i-tiles for shared slice


def host_routing(x, gate_w, e_bias):
    """fp32 numpy replica of reference _routing_weights -> dense [T, E]."""
    logits = (x @ gate_w.T).astype(np.float32)
    scores = (1.0 / (1.0 + np.exp(-logits.astype(np.float32)))).astype(np.float32)
    swb = scores + e_bias[None, :].astype(np.float32)
    t, e = swb.shape
    gsz = e // N_GROUP
    grp = swb.reshape(t, N_GROUP, gsz)
    # top-2 sum per group (values only; ties irrelevant for a sum)
    top2 = np.sort(grp, axis=-1)[:, :, -2:]
    gscores = top2.sum(-1, dtype=np.float32)
    # top TOPK_GROUP groups, lowest-index-first on ties like jax.lax.top_k
    gidx = np.argsort(-gscores, axis=-1, kind="stable")[:, :TOPK_GROUP]
    gmask = np.zeros((t, N_GROUP), bool)
    np.put_along_axis(gmask, gidx, True, axis=1)
    emask = np.repeat(gmask, gsz, axis=1)
    masked = np.where(emask, swb, -np.inf)
    idx = np.argsort(-masked, axis=-1, kind="stable")[:, :TOP_K]
    w = np.take_along_axis(scores, idx, axis=1)
    w = (w / (w.sum(-1, keepdims=True) + 1e-20) * ROUTED_SCALE).astype(np.float32)
    wfull = np.zeros((t, e), np.float32)
    np.put_along_axis(wfull, idx, w, axis=1)
    return wfull


def build_nc(t_total=T, n_exp=EL, with_shared=True):
    """Emit the per-core Bass program (SPMD; per-core data differs, code same)."""
    ntb = t_total // TB
    nc = bacc.Bacc("TRN2", target_bir_lowering=False)

    xT_d = nc.dram_tensor("xT", [H, t_total], F32R, kind="ExternalInput")
    w13_d = nc.dram_tensor("w13", [n_exp, H, IT, 256], F32R, kind="ExternalInput")
    w2t_d = nc.dram_tensor("w2t", [n_exp, I, H], F32R, kind="ExternalInput")
    colwb_d = nc.dram_tensor("colwb", [n_exp, 128, t_total], F32, kind="ExternalInput")
    if with_shared:
        sw13_d = nc.dram_tensor("sw13", [H, IST, 256], F32R, kind="ExternalInput")
        sw2t_d = nc.dram_tensor("sw2t", [ISL, H], F32R, kind="ExternalInput")
    yT_d = nc.dram_tensor("yT", [H, t_total], F32, kind="ExternalOutput")

    with tile.TileContext(nc) as tc:
        with (
            tc.tile_pool(name="xp", bufs=1) as xp,
            tc.tile_pool(name="wp", bufs=6) as wp,
            tc.tile_pool(name="w2p", bufs=16) as w2p,
            tc.tile_pool(name="hp", bufs=1) as hp,
            tc.tile_pool(name="cp", bufs=2) as cp,
            tc.tile_pool(name="op", bufs=1) as op,
            tc.tile_pool(name="sp", bufs=3) as sp,
            tc.tile_pool(name="ps", bufs=2, space="PSUM") as ps,
        ):
            for tb in range(ntb):
                t0 = tb * TB
                # --- load x block [H, TB] as 16 tiles in one wide sbuf tile
                x_sb = xp.tile([128, HT * TB], F32R, tag="x")
                for h in range(HT):
                    nc.sync.dma_start(
                        x_sb[:, bass.ts(h, TB)],
                        xT_d[128 * h : 128 * (h + 1), t0 : t0 + TB],
                    )

                out_sb = op.tile([128, HT * TB], F32, tag="out")

                n_units = n_exp + (1 if with_shared else 0)
                for j in range(n_units):
                    shared = j == n_exp
                    n_it = IST if shared else IT

                    # --- combine weights for this expert (pre-broadcast on host)
                    if not shared:
                        col_sb = cp.tile([128, TB], F32, tag="col")
                        nc.sync.dma_start(col_sb[:], colwb_d[j, :, t0 : t0 + TB])

                    # --- phase C: hT = silu(w1@xT) * (w3@xT) [* colw]
                    h_sb = hp.tile([128, IT * TB], F32R, tag="h")
                    for ig in range(n_it):
                        g_ps = ps.tile([128, TB], F32, tag="g")
                        u_ps = ps.tile([128, TB], F32, tag="u")
                        for h in range(HT):
                            w13_sb = wp.tile([128, 256], F32R, tag="w13")
                            src = (
                                sw13_d[128 * h : 128 * (h + 1), ig, :]
                                if shared
                                else w13_d[j, 128 * h : 128 * (h + 1), ig, :]
                            )
                            nc.sync.dma_start(w13_sb[:], src)
                            rhs = x_sb[:, bass.ts(h, TB)]
                            nc.tensor.matmul(
                                g_ps[:], w13_sb[:, 0:128], rhs,
                                start=(h == 0), stop=(h == HT - 1),
                            )
                            nc.tensor.matmul(
                                u_ps[:], w13_sb[:, 128:256], rhs,
                                start=(h == 0), stop=(h == HT - 1),
                            )
                        silu_sb = sp.tile([128, TB], F32, tag="silu")
                        nc.scalar.activation(silu_sb[:], g_ps[:], AF.Silu)
                        hslice = h_sb[:, bass.ts(ig, TB)]
                        if shared:
                            nc.vector.tensor_mul(hslice, u_ps[:], silu_sb[:])
                        else:
                            nc.vector.tensor_mul(hslice, u_ps[:], col_sb[:])
                            nc.vector.tensor_mul(hslice, hslice, silu_sb[:])

                    # --- phase D: out += w2^T-tiles @ hT, 2 h'-tiles per group
                    for hg in range(HT // 2):
                        w2_sb = []
                        for i in range(n_it):
                            w2c = w2p.tile([128, 256], F32R, tag="w2")
                            src = (
                                sw2t_d[128 * i : 128 * (i + 1),
                                       256 * hg : 256 * (hg + 1)]
                                if shared
                                else w2t_d[j, 128 * i : 128 * (i + 1),
                                           256 * hg : 256 * (hg + 1)]
                            )
                            nc.sync.dma_start(w2c[:], src)
                            w2_sb.append(w2c)
                        for hl in range(2):
                            o_ps = ps.tile([128, TB], F32, tag="o")
                            for i in range(n_it):
                                nc.tensor.matmul(
                                    o_ps[:],
                                    w2_sb[i][:, bass.ts(hl, 128)],
                                    h_sb[:, bass.ts(i, TB)],
                                    start=(i == 0), stop=(i == n_it - 1),
                                )
                            oslice = out_sb[:, bass.ts(2 * hg + hl, TB)]
                            if j == 0:
                                nc.vector.tensor_copy(oslice, o_ps[:])
                            else:
                                nc.vector.tensor_add(oslice, oslice, o_ps[:])

                # --- write back this token block
                for h in range(HT):
                    nc.sync.dma_start(
                        yT_d[128 * h : 128 * (h + 1), t0 : t0 + TB],
                        out_sb[:, bass.ts(h, TB)],
                    )
    nc.compile()
    return nc


def prep_inputs(hidden_states, gate_w, e_bias, w1, w3, w2, sw1, sw3, sw2,
                n_cores=N_CORES):
    """Host-side sharding: returns per-core in_maps."""
    x = np.asarray(hidden_states, np.float32)
    t_total = x.shape[0]
    xT = np.ascontiguousarray(x.T)
    wfull = host_routing(x, np.asarray(gate_w, np.float32),
                         np.asarray(e_bias, np.float32))

    w1t = np.asarray(w1, np.float32).transpose(0, 2, 1)  # [E, H, I]
    w3t = np.asarray(w3, np.float32).transpose(0, 2, 1)
    n_exp_total = w1t.shape[0]
    w13 = np.empty((n_exp_total, H, IT, 256), np.float32)
    w13[..., 0:128] = np.ascontiguousarray(w1t).reshape(n_exp_total, H, IT, 128)
    w13[..., 128:256] = np.ascontiguousarray(w3t).reshape(n_exp_total, H, IT, 128)
    w2t = np.ascontiguousarray(np.asarray(w2, np.float32).transpose(0, 2, 1))

    sw1t = np.ascontiguousarray(np.asarray(sw1, np.float32).T)  # [H, IS]
    sw3t = np.ascontiguousarray(np.asarray(sw3, np.float32).T)
    sw2_f = np.asarray(sw2, np.float32)                          # [H, IS]

    in_maps = []
    for c in range(n_cores):
        el0 = c * EL
        sl0 = c * ISL
        sw13 = np.empty((H, IST, 256), np.float32)
        for ig in range(IST):
            sw13[:, ig, 0:128] = sw1t[:, sl0 + 128 * ig : sl0 + 128 * (ig + 1)]
            sw13[:, ig, 128:256] = sw3t[:, sl0 + 128 * ig : sl0 + 128 * (ig + 1)]
        sw2t = np.ascontiguousarray(sw2_f[:, sl0 : sl0 + ISL].T)  # [ISL, H]
        colwb = np.ascontiguousarray(
            np.broadcast_to(
                wfull[:, el0 : el0 + EL].T[:, None, :], (EL, 128, t_total)
            )
        )
        in_maps.append({
            "xT": xT,
            "w13": np.ascontiguousarray(w13[el0 : el0 + EL]),
            "w2t": np.ascontiguousarray(w2t[el0 : el0 + EL]),
            "colwb": colwb,
            "sw13": sw13,
            "sw2t": sw2t,
        })
    return in_maps


_NC_CACHE = {}


def run(inputs, trace=False):
    t_total = inputs["hidden_states"].shape[0]
    key = t_total
    if key not in _NC_CACHE:
        _NC_CACHE[key] = build_nc(t_total=t_total)
    nc = _NC_CACHE[key]
    in_maps = prep_inputs(**inputs)
    res = run_bass_kernel_spmd(nc, in_maps, core_ids=list(range(N_CORES)),
                               trace=trace)
    acc = res.results[0]["yT"].astype(np.float32).copy()
    for c in range(1, N_CORES):
        acc += res.results[c]["yT"]
    return np.ascontiguousarray(acc.T), res


def kernel(**inputs) -> np.ndarray:
    out, _ = run(inputs, trace=False)
    return out
